# revision 2
# baseline (speedup 1.0000x reference)
"""CAM (channel attention) kernel for Trainium2, SPMD over 8 NeuronCores.

Full inputs: x [16, 512, 64, 64] f32, gamma [1] f32.
Math per batch b (N = 64*64 = 4096 pixels, C = 512 channels):
    q = x[b].reshape(C, N)
    E = q @ q.T                            # (C, C)
    A = softmax(rowmax(E) - E, axis=-1)    # == exp(rowmin(E) - E) / rowsum
    y[b] = gamma * (A @ q) + x[b]

Sharding: data-parallel over batch. Each core takes 2 of the 16 batch
elements; no cross-core communication.

Key structure (vs a straightforward implementation):
  - x is cast fp32->bf16 during the load DMA (SWDGE cast) -> qn; the
    residual add uses qn (bf16) instead of fp32 x, freeing the compute
    engines from the cast and SBUF from the fp32 copy.
  - E is symmetric: only the upper-triangle chunk blocks are computed
    (rhs width 512/384/256/128 for chunk rows 0..3); the lower blocks are
    mirrored with 6 fp32 PE transposes. Bit-exact vs computing full E.
  - q^T for the Gram matmul comes from PE-transposes (batch 0, while DMA
    is loading) and xbar DMA-transposes (batch 1, while PE runs batch 0's
    output phase).
  - Matmuls in bf16 with fp32 PSUM accumulation; softmax and residual in
    fp32.
"""

from contextlib import ExitStack

import numpy as np

import concourse.bacc as bacc
import concourse.bass as bass
import concourse.mybir as mybir
import concourse.tile as tile
from concourse.bass_utils import run_bass_kernel_spmd
from concourse.masks import make_identity

P = 128            # SBUF partitions
C = 512            # channels
CT = C // P        # 4 channel chunks
NPIX = 4096        # H*W
SL = 512           # pixel-slice width
NS = NPIX // SL    # 8 pixel slices
KT = NPIX // P     # 32 contraction chunks for E
MB = 2             # batch elements per core
NCORES = 8

# per-batch q^T strategy: "pe" (TensorE transpose) or "xbar" (DMA transpose)
TRANS_MODE = ("pe", "xbar")

F32 = mybir.dt.float32
BF16 = mybir.dt.bfloat16
AX = mybir.AxisListType.X
MIN = mybir.AluOpType.min
EXP = mybir.ActivationFunctionType.Exp


def build_nc() -> bacc.Bacc:
    nc = bacc.Bacc("TRN2", target_bir_lowering=False, debug=False)
    x = nc.declare_dram_parameter("x", [MB, C, 64, 64], F32, isOutput=False)
    g = nc.declare_dram_parameter("gamma", [1], F32, isOutput=False)
    y = nc.declare_dram_parameter("y", [MB, C, 64, 64], F32, isOutput=True)

    # [b, p, t, n]: channel = t*128 + p, pixel = n
    xr = x[:].rearrange("b (t p) h w -> b p t (h w)", p=P)
    yr = y[:].rearrange("b (t p) h w -> b p t (h w)", p=P)

    with tile.TileContext(nc) as tc, ExitStack() as ctx:
        qnpool = ctx.enter_context(tc.tile_pool(name="qn", bufs=2))
        qtpool = ctx.enter_context(tc.tile_pool(name="qt", bufs=2))
        epool = ctx.enter_context(tc.tile_pool(name="esb", bufs=2))
        apool = ctx.enter_context(tc.tile_pool(name="a", bufs=2))
        atpool = ctx.enter_context(tc.tile_pool(name="at", bufs=2))
        upool = ctx.enter_context(tc.tile_pool(name="u", bufs=3))
        ypool = ctx.enter_context(tc.tile_pool(name="y", bufs=3))
        stat = ctx.enter_context(tc.tile_pool(name="stat", bufs=16))
        cpool = ctx.enter_context(tc.tile_pool(name="const", bufs=1))
        epsum = ctx.enter_context(tc.tile_pool(name="eps", bufs=1, space="PSUM"))
        tpsum = ctx.enter_context(tc.tile_pool(name="tps", bufs=2, space="PSUM"))
        opsum = ctx.enter_context(tc.tile_pool(name="ops", bufs=2, space="PSUM"))

        gamma_b = cpool.tile([P, 1], F32)
        nc.gpsimd.dma_start(gamma_b[:], g[:].to_broadcast((P, 1)))
        ident = cpool.tile([P, P], BF16)
        make_identity(nc, ident[:])
        identf = cpool.tile([P, P], F32)
        make_identity(nc, identf[:])

        st = [dict() for _ in range(MB)]

        def alloc_batch(b):
            s = st[b]
            # qn[p, t, n] = bf16(x[b, t*128+p, n])
            s["qn"] = qnpool.tile([P, CT, NPIX], BF16, tag="qn", name="qn")
            # qt[p, t, k, c] = qn[c, t, k*128+p]  (q^T in 128-pixel chunks)
            s["qt"] = qtpool.tile([P, CT, KT, P], BF16, tag="qt", name="qt")
            # full E rows in SBUF: esb[p, m, d] = E[m*128+p, d]
            s["esb"] = epool.tile([P, CT, C], F32, tag="esb", name="esb")
            s["a"] = apool.tile([P, CT, C], BF16, tag="a", name="a")
            # at[p, k, c] = A[c, k*128+p]  (A^T, lhsT for the output matmul)
            s["at"] = atpool.tile([P, CT, C], BF16, tag="at", name="at")
            # upper-triangle E chunk rows; [P, 512] tiles keep matmul
            # targets bank-aligned, row m uses [:, 0:512-128m]
            s["eps"] = [
                epsum.tile([P, C], F32, tag=f"e{m}", name=f"eps{m}")
                for m in range(CT)
            ]

        def load_slice(b, ns):
            """Cast-DMA pixel-slice ns of batch b: HBM fp32 -> SBUF bf16."""
            nc.gpsimd.dma_start(
                st[b]["qn"][:, :, ns * SL:(ns + 1) * SL],
                xr[b, :, :, ns * SL:(ns + 1) * SL],
            )

        def trans_slice(b, ns):
            """Build q^T for pixel-slice ns (4 k-chunks) of batch b."""
            s = st[b]
            for t in range(CT):
                dst = s["qt"][:, t, 4 * ns:4 * ns + 4, :]
                if TRANS_MODE[b] == "xbar":
                    nc.sync.dma_start(
                        dst, s["qn"][:, t, ns * SL:(ns + 1) * SL],
                        transpose=True,
                    )
                else:
                    tp = tpsum.tile([P, SL], BF16, tag="tp", name="tp")
                    for kk in range(4):
                        nc.tensor.transpose(
                            tp[:, kk * P:(kk + 1) * P],
                            s["qn"][:, t,
                                    ns * SL + kk * P:ns * SL + (kk + 1) * P],
                            ident[:],
                        )
                    if t % 2 == 0:
                        nc.vector.tensor_copy(dst, tp[:])
                    else:
                        nc.scalar.copy(dst, tp[:])

        def e_slice(b, ns):
            """E-accumulation matmuls (upper-triangle chunk rows) for the
            4 pixel-chunks of slice ns."""
            s = st[b]
            qt = s["qt"]
            for kk in range(4):
                k = 4 * ns + kk
                for m in range(CT):
                    nc.tensor.matmul(
                        s["eps"][m][:, 0:C - m * P],
                        qt[:, m, k, :],
                        qt[:, m:, k, :],
                        start=(k == 0),
                        stop=(k == KT - 1),
                    )

        def mirror_softmax(b):
            """Mirror E's lower blocks, softmax rows, build A^T."""
            s = st[b]
            esb, a, at = s["esb"], s["a"], s["at"]
            # upper-triangle rows PSUM -> SBUF
            for m in range(CT):
                dst = esb[:, m, m * P:C]
                src = s["eps"][m][:, 0:C - m * P]
                if m % 2 == 0:
                    nc.vector.tensor_copy(dst, src)
                else:
                    nc.scalar.copy(dst, src)
            # lower blocks = transpose of upper blocks (E symmetric)
            for m in range(1, CT):
                for d in range(m):
                    mp = tpsum.tile([P, P], F32, tag="tp", name="mp")
                    nc.tensor.transpose(
                        mp[:], esb[:, d, m * P:(m + 1) * P], identf[:]
                    )
                    nc.vector.tensor_copy(esb[:, m, d * P:(d + 1) * P], mp[:])
            # A = gamma * exp(rowmin - E) / rowsum, row-chunk at a time
            for m in range(CT):
                mn = stat.tile([P, 1], F32, tag="mn", name="mn")
                nc.vector.tensor_reduce(mn[:], esb[:, m, :], AX, MIN)
                u = upool.tile([P, C], F32, tag="u", name="u")
                sm = stat.tile([P, 1], F32, tag="sm", name="sm")
                nc.scalar.activation(
                    u[:], esb[:, m, :], EXP, bias=mn[:], scale=-1.0,
                    accum_out=sm[:],
                )
                rc = stat.tile([P, 1], F32, tag="rc", name="rc")
                nc.vector.reciprocal(rc[:], sm[:])
                sc = stat.tile([P, 1], F32, tag="sc", name="sc")
                nc.vector.tensor_scalar_mul(sc[:], rc[:], gamma_b[:])
                nc.vector.tensor_scalar_mul(a[:, m, :], u[:], sc[:])
            # A^T via PE transposes
            for m in range(CT):
                tp2 = tpsum.tile([P, C], BF16, tag="tp", name="tp2")
                for kk in range(CT):
                    nc.tensor.transpose(
                        tp2[:, kk * P:(kk + 1) * P],
                        a[:, m, kk * P:(kk + 1) * P],
                        ident[:],
                    )
                nc.scalar.copy(at[:, :, m * P:(m + 1) * P], tp2[:])

        def out_slice(b, ns):
            """out = A @ q for pixel-slice ns; residual add; store."""
            s = st[b]
            yt = ypool.tile([P, CT, SL], F32, tag="y", name="yt")
            for m in range(CT):
                ops = opsum.tile([P, SL], F32, tag="o", name="ops")
                for k in range(CT):
                    nc.tensor.matmul(
                        ops[:],
                        s["at"][:, k, m * P:(m + 1) * P],
                        s["qn"][:, k, ns * SL:(ns + 1) * SL],
                        start=(k == 0),
                        stop=(k == CT - 1),
                    )
                nc.vector.tensor_add(
                    yt[:, m, :], ops[:], s["qn"][:, m, ns * SL:(ns + 1) * SL]
                )
            nc.scalar.dma_start(yr[b, :, :, ns * SL:(ns + 1) * SL], yt[:])

        alloc_batch(0)
        alloc_batch(1)

        # ---- batch 0 input phase (E lags the load/transpose by 1 slice) ----
        for ns in range(NS):
            load_slice(0, ns)
            trans_slice(0, ns)
            if ns > 0:
                e_slice(0, ns - 1)
        e_slice(0, NS - 1)

        # batch 1's first slice keeps DMA busy during batch 0's softmax
        load_slice(1, 0)
        trans_slice(1, 0)
        mirror_softmax(0)

        # ---- interleave: batch 0 output phase + batch 1 input phase ----
        for j in range(NS):
            out_slice(0, j)
            if j + 1 < NS:
                load_slice(1, j + 1)
                trans_slice(1, j + 1)
            if j > 0:
                e_slice(1, j - 1)
        e_slice(1, NS - 1)

        mirror_softmax(1)
        for ns in range(NS):
            out_slice(1, ns)

    return nc


_NC = None


def _get_nc() -> bacc.Bacc:
    global _NC
    if _NC is None:
        _NC = build_nc()
        _NC.finalize()
    return _NC


def _run(x: np.ndarray, gamma: np.ndarray, trace: bool = False):
    x = np.ascontiguousarray(x, dtype=np.float32)
    gamma = np.ascontiguousarray(gamma, dtype=np.float32).reshape(1)
    in_maps = [
        {"x": x[MB * i:MB * (i + 1)], "gamma": gamma} for i in range(NCORES)
    ]
    res = run_bass_kernel_spmd(
        _get_nc(), in_maps, core_ids=list(range(NCORES)), trace=trace
    )
    out = np.concatenate([r["y"] for r in res.results], axis=0)
    return out.astype(np.float32, copy=False), res


def kernel(x: np.ndarray, gamma: np.ndarray) -> np.ndarray:
    out, _ = _run(x, gamma, trace=False)
    return out


def kernel_profiled(x: np.ndarray, gamma: np.ndarray):
    out, res = _run(x, gamma, trace=True)
    return out, res


# revision 4
# speedup vs baseline: 1.5064x; 1.5064x over previous
"""CAM (channel attention) kernel for Trainium2, SPMD over 8 NeuronCores.

Full inputs: x [16, 512, 64, 64] f32, gamma [1] f32.
Math per batch b (N = 64*64 = 4096 pixels, C = 512 channels):
    q = x[b].reshape(C, N)
    E = q @ q.T                            # (C, C)
    A = softmax(rowmax(E) - E, axis=-1)    # == exp(rowmin(E) - E) / rowsum
    y[b] = gamma * (A @ q) + x[b]

Sharding: data-parallel over batch. Each core takes 2 of the 16 batch
elements; no cross-core communication.

Key structure (vs a straightforward implementation):
  - x is cast fp32->bf16 during the load DMA (SWDGE cast) -> qn; the
    residual add uses qn (bf16) instead of fp32 x, freeing the compute
    engines from the cast and SBUF from the fp32 copy.
  - E is symmetric: only the upper-triangle chunk blocks are computed
    (rhs width 512/384/256/128 for chunk rows 0..3); the lower blocks are
    mirrored with 6 fp32 PE transposes. Bit-exact vs computing full E.
  - q^T comes from PE transposes interleaved with the E matmuls.
  - The output phase runs per channel-chunk row m: softmax row m ->
    A^T row m -> out row m, so the first output matmuls start ~2us after
    the E phase ends.
  - Matmuls in bf16 with fp32 PSUM accumulation; softmax and residual in
    fp32.
"""

from contextlib import ExitStack

import numpy as np

import concourse.bacc as bacc
import concourse.bass as bass
import concourse.mybir as mybir
import concourse.tile as tile
from concourse.bass_utils import run_bass_kernel_spmd
from concourse.masks import make_identity

P = 128            # SBUF partitions
C = 512            # channels
CT = C // P        # 4 channel chunks
NPIX = 4096        # H*W
SL = 512           # pixel-slice width
NS = NPIX // SL    # 8 pixel slices
KT = NPIX // P     # 32 contraction chunks for E
MB = 2             # batch elements per core
NCORES = 8

F32 = mybir.dt.float32
BF16 = mybir.dt.bfloat16
AX = mybir.AxisListType.X
MIN = mybir.AluOpType.min
EXP = mybir.ActivationFunctionType.Exp


def build_nc() -> bacc.Bacc:
    nc = bacc.Bacc("TRN2", target_bir_lowering=False, debug=False)
    x = nc.declare_dram_parameter("x", [MB, C, 64, 64], F32, isOutput=False)
    g = nc.declare_dram_parameter("gamma", [1], F32, isOutput=False)
    y = nc.declare_dram_parameter("y", [MB, C, 64, 64], F32, isOutput=True)

    # [b, p, t, n]: channel = t*128 + p, pixel = n
    xr = x[:].rearrange("b (t p) h w -> b p t (h w)", p=P)
    yr = y[:].rearrange("b (t p) h w -> b p t (h w)", p=P)

    with tile.TileContext(nc) as tc, ExitStack() as ctx:
        qnpool = ctx.enter_context(tc.tile_pool(name="qn", bufs=2))
        qtpool = ctx.enter_context(tc.tile_pool(name="qt", bufs=2))
        epool = ctx.enter_context(tc.tile_pool(name="esb", bufs=2))
        apool = ctx.enter_context(tc.tile_pool(name="a", bufs=2))
        atpool = ctx.enter_context(tc.tile_pool(name="at", bufs=2))
        upool = ctx.enter_context(tc.tile_pool(name="u", bufs=3))
        ypool = ctx.enter_context(tc.tile_pool(name="y", bufs=6))
        stat = ctx.enter_context(tc.tile_pool(name="stat", bufs=16))
        cpool = ctx.enter_context(tc.tile_pool(name="const", bufs=1))
        epsum = ctx.enter_context(tc.tile_pool(name="eps", bufs=1, space="PSUM"))
        tpsum = ctx.enter_context(tc.tile_pool(name="tps", bufs=2, space="PSUM"))
        opsum = ctx.enter_context(tc.tile_pool(name="ops", bufs=2, space="PSUM"))

        st = [dict() for _ in range(MB)]

        def alloc_batch(b):
            s = st[b]
            # qn[p, t, n] = bf16(x[b, t*128+p, n])
            s["qn"] = qnpool.tile([P, CT, NPIX], BF16, tag="qn", name="qn")
            # qt[p, t, k, c] = qn[c, t, k*128+p]  (q^T in 128-pixel chunks)
            s["qt"] = qtpool.tile([P, CT, KT, P], BF16, tag="qt", name="qt")
            # full E rows in SBUF: esb[p, m, d] = E[m*128+p, d]
            s["esb"] = epool.tile([P, CT, C], F32, tag="esb", name="esb")
            s["a"] = apool.tile([P, CT, C], BF16, tag="a", name="a")
            # at[p, k, c] = A[c, k*128+p]  (A^T, lhsT for the output matmul)
            s["at"] = atpool.tile([P, CT, C], BF16, tag="at", name="at")
            # upper-triangle E chunk rows; [P, 512] tiles keep matmul
            # targets bank-aligned, row m uses [:, 0:512-128m]
            s["eps"] = [
                epsum.tile([P, C], F32, tag=f"e{m}", name=f"eps{m}")
                for m in range(CT)
            ]

        def load_slice(b, ns, split=False):
            """Cast-DMA pixel-slice ns of batch b: HBM fp32 -> SBUF bf16."""
            if split:  # per-channel-chunk DMAs so the first transpose can
                for t in range(CT):  # start after 1/4 of the slice
                    nc.gpsimd.dma_start(
                        st[b]["qn"][:, t, ns * SL:(ns + 1) * SL],
                        xr[b, :, t, ns * SL:(ns + 1) * SL],
                    )
            else:
                nc.gpsimd.dma_start(
                    st[b]["qn"][:, :, ns * SL:(ns + 1) * SL],
                    xr[b, :, :, ns * SL:(ns + 1) * SL],
                )

        def trans_slice(b, ns):
            """Build q^T for pixel-slice ns (4 k-chunks) via PE."""
            s = st[b]
            for t in range(CT):
                dst = s["qt"][:, t, 4 * ns:4 * ns + 4, :]
                tp = tpsum.tile([P, SL], BF16, tag="tp", name="tp")
                for kk in range(4):
                    nc.tensor.transpose(
                        tp[:, kk * P:(kk + 1) * P],
                        s["qn"][:, t, ns * SL + kk * P:ns * SL + (kk + 1) * P],
                        ident[:],
                    )
                if t % 2 == 0:
                    nc.vector.tensor_copy(dst, tp[:])
                else:
                    nc.scalar.copy(dst, tp[:])

        def e_slice(b, ns):
            """E-accumulation matmuls (upper-triangle chunk rows) for the
            4 pixel-chunks of slice ns."""
            s = st[b]
            qt = s["qt"]
            for kk in range(4):
                k = 4 * ns + kk
                for m in range(CT):
                    nc.tensor.matmul(
                        s["eps"][m][:, 0:C - m * P],
                        qt[:, m, k, :],
                        qt[:, m:, k, :],
                        start=(k == 0),
                        stop=(k == KT - 1),
                    )

        def mirror(b):
            """E rows PSUM -> SBUF; mirror lower blocks (E symmetric)."""
            s = st[b]
            esb = s["esb"]
            for m in range(CT):
                dst = esb[:, m, m * P:C]
                src = s["eps"][m][:, 0:C - m * P]
                if m % 2 == 0:
                    nc.vector.tensor_copy(dst, src)
                else:
                    nc.scalar.copy(dst, src)
            for m in range(1, CT):
                for d in range(m):
                    mp = tpsum.tile([P, P], F32, tag="tp", name="mp")
                    nc.tensor.transpose(
                        mp[:], esb[:, d, m * P:(m + 1) * P], identf[:]
                    )
                    nc.vector.tensor_copy(esb[:, m, d * P:(d + 1) * P], mp[:])

        def softmax_row(b, m):
            """A row-chunk m = gamma * exp(rowmin - E) / rowsum; build A^T."""
            s = st[b]
            esb, a, at = s["esb"], s["a"], s["at"]
            mn = stat.tile([P, 1], F32, tag="mn", name="mn")
            nc.vector.tensor_reduce(mn[:], esb[:, m, :], AX, MIN)
            u = upool.tile([P, C], F32, tag="u", name="u")
            sm = stat.tile([P, 1], F32, tag="sm", name="sm")
            nc.scalar.activation(
                u[:], esb[:, m, :], EXP, bias=mn[:], scale=-1.0,
                accum_out=sm[:],
            )
            rc = stat.tile([P, 1], F32, tag="rc", name="rc")
            nc.vector.reciprocal(rc[:], sm[:])
            sc = stat.tile([P, 1], F32, tag="sc", name="sc")
            nc.vector.tensor_scalar_mul(sc[:], rc[:], gamma_b[:])
            nc.vector.tensor_scalar_mul(a[:, m, :], u[:], sc[:])
            tp2 = tpsum.tile([P, C], BF16, tag="tp", name="tp2")
            for kk in range(CT):
                nc.tensor.transpose(
                    tp2[:, kk * P:(kk + 1) * P],
                    a[:, m, kk * P:(kk + 1) * P],
                    ident[:],
                )
            nc.scalar.copy(at[:, :, m * P:(m + 1) * P], tp2[:])

        def out_row(b, m, ns):
            """out row-chunk m for pixel-slice ns; residual add; store."""
            s = st[b]
            ops = opsum.tile([P, SL], F32, tag="o", name="ops")
            for k in range(CT):
                nc.tensor.matmul(
                    ops[:],
                    s["at"][:, k, m * P:(m + 1) * P],
                    s["qn"][:, k, ns * SL:(ns + 1) * SL],
                    start=(k == 0),
                    stop=(k == CT - 1),
                )
            yt = ypool.tile([P, SL], F32, tag="y", name="yt")
            nc.vector.tensor_add(
                yt[:], ops[:], s["qn"][:, m, ns * SL:(ns + 1) * SL]
            )
            nc.scalar.dma_start(yr[b, :, m, ns * SL:(ns + 1) * SL], yt[:])

        alloc_batch(0)
        alloc_batch(1)

        # first load goes out before the const setup so DMA starts ASAP
        load_slice(0, 0, split=True)

        gamma_b = cpool.tile([P, 1], F32)
        nc.gpsimd.dma_start(gamma_b[:], g[:].to_broadcast((P, 1)))
        ident = cpool.tile([P, P], BF16)
        make_identity(nc, ident[:])
        identf = cpool.tile([P, P], F32)
        make_identity(nc, identf[:])

        # ---- batch 0 input phase (E lags the load/transpose by 1 slice) ----
        trans_slice(0, 0)
        for ns in range(1, NS):
            load_slice(0, ns)
            trans_slice(0, ns)
            e_slice(0, ns - 1)
        e_slice(0, NS - 1)

        # batch 1's first loads + transpose fill PE/DMA during batch 0's
        # mirror + first softmax row
        load_slice(1, 0)
        load_slice(1, 1)
        trans_slice(1, 0)
        mirror(0)

        # ---- batch 0 output phase (per row-chunk) + batch 1 input phase ----
        # batch-1 input work units, emitted eagerly ahead of output rows
        def w_unit(k):
            if k + 2 < NS:
                load_slice(1, k + 2)
            if k + 1 < NS:
                trans_slice(1, k + 1)
            e_slice(1, k)

        w_unit(0)
        for m in range(CT):
            softmax_row(0, m)
            w_unit(2 * m + 1)
            for ns in range(NS):
                out_row(0, m, ns)
                if ns == 3 and 2 * m + 2 < NS:
                    w_unit(2 * m + 2)

        mirror(1)
        for m in range(CT):
            softmax_row(1, m)
            for ns in range(NS):
                out_row(1, m, ns)

    return nc


_NC = None


def _get_nc() -> bacc.Bacc:
    global _NC
    if _NC is None:
        _NC = build_nc()
        _NC.finalize()
    return _NC


def _run(x: np.ndarray, gamma: np.ndarray, trace: bool = False):
    x = np.ascontiguousarray(x, dtype=np.float32)
    gamma = np.ascontiguousarray(gamma, dtype=np.float32).reshape(1)
    in_maps = [
        {"x": x[MB * i:MB * (i + 1)], "gamma": gamma} for i in range(NCORES)
    ]
    res = run_bass_kernel_spmd(
        _get_nc(), in_maps, core_ids=list(range(NCORES)), trace=trace
    )
    out = np.concatenate([r["y"] for r in res.results], axis=0)
    return out.astype(np.float32, copy=False), res


def kernel(x: np.ndarray, gamma: np.ndarray) -> np.ndarray:
    out, _ = _run(x, gamma, trace=False)
    return out


def kernel_profiled(x: np.ndarray, gamma: np.ndarray):
    out, res = _run(x, gamma, trace=True)
    return out, res


# revision 14
# speedup vs baseline: 1.6075x; 1.0671x over previous
"""CAM (channel attention) kernel for Trainium2, SPMD over 8 NeuronCores.

Full inputs: x [16, 512, 64, 64] f32, gamma [1] f32.
Math per batch b (N = 64*64 = 4096 pixels, C = 512 channels):
    q = x[b].reshape(C, N)
    E = q @ q.T                            # (C, C)
    A = softmax(rowmax(E) - E, axis=-1)    # == exp(rowmin(E) - E) / rowsum
    y[b] = gamma * (A @ q) + x[b]

Sharding: data-parallel over batch. Each core takes 2 of the 16 batch
elements; no cross-core communication.

Key structure (vs a straightforward implementation):
  - x is cast fp32->bf16 during the load DMA (SWDGE cast) -> qn; the
    residual add uses qn (bf16) instead of fp32 x, freeing the compute
    engines from the cast and SBUF from the fp32 copy.
  - E is symmetric: only the upper-triangle chunk blocks are computed
    (rhs width 512/384/256/128 for chunk rows 0..3); the lower blocks are
    mirrored with 6 fp32 PE transposes. Bit-exact vs computing full E.
  - q^T comes from PE transposes interleaved with the E matmuls.
  - The output phase runs per channel-chunk row m: softmax row m ->
    A^T row m -> out row m, so the first output matmuls start ~2us after
    the E phase ends.
  - Matmuls in bf16 with fp32 PSUM accumulation; softmax and residual in
    fp32.
"""

from contextlib import ExitStack

import numpy as np

import concourse.bacc as bacc
import concourse.bass as bass
import concourse.mybir as mybir
import concourse.tile as tile
from concourse.bass_utils import run_bass_kernel_spmd
from concourse.masks import make_identity

P = 128            # SBUF partitions
C = 512            # channels
CT = C // P        # 4 channel chunks
NPIX = 4096        # H*W
SL = 512           # pixel-slice width
NS = NPIX // SL    # 8 pixel slices
KT = NPIX // P     # 32 contraction chunks for E
MB = 2             # batch elements per core
NCORES = 8

F32 = mybir.dt.float32
BF16 = mybir.dt.bfloat16
FP8 = mybir.dt.float8e4
AX = mybir.AxisListType.X
MIN = mybir.AluOpType.min
MULT = mybir.AluOpType.mult
ADD = mybir.AluOpType.add
EXP = mybir.ActivationFunctionType.Exp
DR = mybir.MatmulPerfMode.DoubleRow


def build_nc() -> bacc.Bacc:
    nc = bacc.Bacc("TRN2", target_bir_lowering=False, debug=False)
    x = nc.declare_dram_parameter("x", [MB, C, 64, 64], F32, isOutput=False)
    g = nc.declare_dram_parameter("gamma", [1], F32, isOutput=False)
    y = nc.declare_dram_parameter("y", [MB, C, 64, 64], F32, isOutput=True)

    # [b, p, t, n]: channel = t*128 + p, pixel = n
    xr = x[:].rearrange("b (t p) h w -> b p t (h w)", p=P)
    yr = y[:].rearrange("b (t p) h w -> b p t (h w)", p=P)

    with tile.TileContext(nc) as tc, ExitStack() as ctx:
        qnpool = ctx.enter_context(tc.tile_pool(name="qn", bufs=2))
        qtpool = ctx.enter_context(tc.tile_pool(name="qt", bufs=2))
        epool = ctx.enter_context(tc.tile_pool(name="esb", bufs=2))
        apool = ctx.enter_context(tc.tile_pool(name="a", bufs=2))
        atpool = ctx.enter_context(tc.tile_pool(name="at", bufs=2))
        upool = ctx.enter_context(tc.tile_pool(name="u", bufs=2))
        ypool = ctx.enter_context(tc.tile_pool(name="y", bufs=3))
        stat = ctx.enter_context(tc.tile_pool(name="stat", bufs=16))
        cpool = ctx.enter_context(tc.tile_pool(name="const", bufs=1))
        epsum = ctx.enter_context(tc.tile_pool(name="eps", bufs=1, space="PSUM"))
        tpsum = ctx.enter_context(tc.tile_pool(name="tps", bufs=2, space="PSUM"))
        opsum = ctx.enter_context(tc.tile_pool(name="ops", bufs=2, space="PSUM"))

        st = [dict() for _ in range(MB)]

        def alloc_batch(b):
            s = st[b]
            # qn[p, t, n] = bf16(x[b, t*128+p, n])
            s["qn"] = qnpool.tile([P, CT, NPIX], BF16, tag="qn", name="qn")
            # qt[p, t, k, c] = qn[c, t, k*128+p]  (q^T in 128-pixel chunks)
            s["qt"] = qtpool.tile([P, CT, KT, P], BF16, tag="qt", name="qt")
            # fp8 copy of qn for the DoubleRow output matmul
            s["qn8"] = qnpool.tile([P, CT, NPIX], FP8, tag="qn8", name="qn8")
            # full E rows in SBUF: esb[p, m, d] = E[m*128+p, d]
            s["esb"] = epool.tile([P, CT, C], F32, tag="esb", name="esb")
            s["a"] = apool.tile([P, CT, C], BF16, tag="a", name="a")
            # at[p, k, c] = A[c, k*128+p]  (A^T, lhsT for the output matmul)
            s["at"] = atpool.tile([P, CT, C], FP8, tag="at", name="at")
            # upper-triangle E chunk rows; [P, 512] tiles keep matmul
            # targets bank-aligned, row m uses [:, 0:512-128m]
            s["eps"] = [
                epsum.tile([P, C], F32, tag=f"e{m}", name=f"eps{m}")
                for m in range(CT)
            ]

        def load_slice(b, ns, split=False):
            """Cast-DMA pixel-slice ns of batch b: HBM fp32 -> SBUF bf16."""
            if split:  # per-channel-chunk DMAs so the first transpose can
                for t in range(CT):  # start after 1/4 of the slice
                    nc.gpsimd.dma_start(
                        st[b]["qn"][:, t, ns * SL:(ns + 1) * SL],
                        xr[b, :, t, ns * SL:(ns + 1) * SL],
                    )
            else:
                nc.gpsimd.dma_start(
                    st[b]["qn"][:, :, ns * SL:(ns + 1) * SL],
                    xr[b, :, :, ns * SL:(ns + 1) * SL],
                )

        def trans_slice(b, ns):
            """Build q^T for pixel-slice ns (4 k-chunks) via PE."""
            s = st[b]
            for t in range(CT):
                dst = s["qt"][:, t, 4 * ns:4 * ns + 4, :]
                tp = tpsum.tile([P, SL], BF16, tag="tp", name="tp")
                for kk in range(4):
                    nc.tensor.transpose(
                        tp[:, kk * P:(kk + 1) * P],
                        s["qn"][:, t, ns * SL + kk * P:ns * SL + (kk + 1) * P],
                        ident[:],
                    )
                if t % 2 == 0:
                    nc.vector.tensor_copy(dst, tp[:])
                else:
                    nc.scalar.copy(dst, tp[:])
            # fp8 copy of the slice for the DoubleRow output matmul
            nc.scalar.copy(
                s["qn8"][:, :, ns * SL:(ns + 1) * SL],
                s["qn"][:, :, ns * SL:(ns + 1) * SL],
            )

        def e_slice(b, ns):
            """E-accumulation matmuls (upper-triangle chunk rows) for the
            4 pixel-chunks of slice ns."""
            s = st[b]
            qt = s["qt"]
            for kk in range(4):
                k = 4 * ns + kk
                for m in range(CT):
                    nc.tensor.matmul(
                        s["eps"][m][:, 0:C - m * P],
                        qt[:, m, k, :],
                        qt[:, m:, k, :],
                        start=(k == 0),
                        stop=(k == KT - 1),
                    )

        def mirror(b):
            """E rows PSUM -> SBUF; mirror lower blocks (E symmetric)."""
            s = st[b]
            esb = s["esb"]
            for m in range(CT):
                dst = esb[:, m, m * P:C]
                src = s["eps"][m][:, 0:C - m * P]
                if m % 2 == 0:
                    nc.vector.tensor_copy(dst, src)
                else:
                    nc.scalar.copy(dst, src)
            for m in range(1, CT):
                for d in range(m):
                    mp = tpsum.tile([P, P], F32, tag="tp", name="mp")
                    nc.tensor.transpose(
                        mp[:], esb[:, d, m * P:(m + 1) * P], identf[:]
                    )
                    nc.vector.tensor_copy(esb[:, m, d * P:(d + 1) * P], mp[:])

        def softmax_row(b, m):
            """A row-chunk m = gamma * exp(rowmin - E) / rowsum; build A^T."""
            s = st[b]
            esb, a, at = s["esb"], s["a"], s["at"]
            mn = stat.tile([P, 1], F32, tag="mn", name="mn")
            nc.vector.tensor_reduce(mn[:], esb[:, m, :], AX, MIN)
            u = upool.tile([P, C], F32, tag="u", name="u")
            sm = stat.tile([P, 1], F32, tag="sm", name="sm")
            nc.scalar.activation(
                u[:], esb[:, m, :], EXP, bias=mn[:], scale=-1.0,
                accum_out=sm[:],
            )
            rc = stat.tile([P, 1], F32, tag="rc", name="rc")
            nc.vector.reciprocal(rc[:], sm[:])
            # gamma is NOT folded into A here (it scales the output instead)
            nc.vector.tensor_scalar_mul(a[:, m, :], u[:], rc[:])
            tp2 = tpsum.tile([P, C], BF16, tag="tp", name="tp2")
            for kk in range(CT):
                nc.tensor.transpose(
                    tp2[:, kk * P:(kk + 1) * P],
                    a[:, m, kk * P:(kk + 1) * P],
                    ident[:],
                )
            nc.scalar.copy(at[:, :, m * P:(m + 1) * P], tp2[:])

        ytiles = {}

        def out_row(b, m, ns):
            """out row-chunk m for pixel-slice ns (fp8 DoubleRow matmuls);
            fused gamma-scale + residual add; paired 1MB stores."""
            s = st[b]
            ops = opsum.tile([P, SL], F32, tag="o", name="ops")
            for kp in range(CT // 2):
                nc.tensor.matmul(
                    ops[:],
                    s["at"][:, 2 * kp:2 * kp + 2, m * P:(m + 1) * P],
                    s["qn8"][:, 2 * kp:2 * kp + 2, ns * SL:(ns + 1) * SL],
                    start=(kp == 0),
                    stop=(kp == CT // 2 - 1),
                    perf_mode=DR,
                )
            if ns % 2 == 0:
                ytiles[b, m] = ypool.tile([P, 2, SL], F32, tag="y", name="yt")
            yt = ytiles[b, m]
            # yt = gamma * out + q  (q bf16; exact-x residual is within tol)
            nc.vector.scalar_tensor_tensor(
                yt[:, ns % 2, :], ops[:], gamma_b[:],
                s["qn"][:, m, ns * SL:(ns + 1) * SL], MULT, ADD,
            )
            if ns % 2 == 1:
                nc.sync.dma_start(
                    yr[b, :, m, (ns - 1) * SL:(ns + 1) * SL], yt[:]
                )

        alloc_batch(0)
        alloc_batch(1)

        # first load goes out before the const setup so DMA starts ASAP
        load_slice(0, 0, split=True)

        gamma_b = cpool.tile([P, 1], F32)
        nc.gpsimd.dma_start(gamma_b[:], g[:].to_broadcast((P, 1)))
        ident = cpool.tile([P, P], BF16)
        make_identity(nc, ident[:])
        identf = cpool.tile([P, P], F32)
        make_identity(nc, identf[:])

        # ---- batch 0 input phase (E lags the load/transpose by 1 slice) ----
        trans_slice(0, 0)
        for ns in range(1, NS):
            load_slice(0, ns)
            trans_slice(0, ns)
            e_slice(0, ns - 1)
        e_slice(0, NS - 1)

        # batch 1's first loads + transpose fill PE/DMA during batch 0's
        # mirror + first softmax row
        load_slice(1, 0)
        load_slice(1, 1)
        trans_slice(1, 0)
        mirror(0)

        # ---- batch 0 output phase (per row-chunk) + batch 1 input phase ----
        # batch-1 input work units, emitted eagerly ahead of output rows
        def w_unit(k):
            if k + 2 < NS:
                load_slice(1, k + 2)
            if k + 1 < NS:
                trans_slice(1, k + 1)
            e_slice(1, k)

        w_unit(0)
        for m in range(CT):
            softmax_row(0, m)
            w_unit(2 * m + 1)
            for ns in range(NS):
                out_row(0, m, ns)
                if ns == 3 and 2 * m + 2 < NS:
                    w_unit(2 * m + 2)

        mirror(1)
        for m in range(CT):
            softmax_row(1, m)
            for ns in range(NS):
                out_row(1, m, ns)

    return nc


_NC = None


def _get_nc() -> bacc.Bacc:
    global _NC
    if _NC is None:
        _NC = build_nc()
        _NC.finalize()
    return _NC


def _run(x: np.ndarray, gamma: np.ndarray, trace: bool = False):
    x = np.ascontiguousarray(x, dtype=np.float32)
    gamma = np.ascontiguousarray(gamma, dtype=np.float32).reshape(1)
    in_maps = [
        {"x": x[MB * i:MB * (i + 1)], "gamma": gamma} for i in range(NCORES)
    ]
    res = run_bass_kernel_spmd(
        _get_nc(), in_maps, core_ids=list(range(NCORES)), trace=trace
    )
    out = np.concatenate([r["y"] for r in res.results], axis=0)
    return out.astype(np.float32, copy=False), res


def kernel(x: np.ndarray, gamma: np.ndarray) -> np.ndarray:
    out, _ = _run(x, gamma, trace=False)
    return out


def kernel_profiled(x: np.ndarray, gamma: np.ndarray):
    out, res = _run(x, gamma, trace=True)
    return out, res
